# revision 27
# baseline (speedup 1.0000x reference)
"""GQA MultiHeadAttention (RoPE, causal) Bass/Tile kernel for 8 Trainium2 cores.

Problem: x[2,2048,2048] @ Wq/Wk/Wv -> RoPE -> causal GQA attention -> @ Wo.
D=2048, H=16 heads, G=4 KV groups, HD=128, B=2, S=2048.

Sharding (SPMD, one program, per-core data):
  core c -> batch b=c//4, KV-group g=c%4 (heads 4g..4g+3).
  Each core: QKV projection for its group from x[b]^T, RoPE, 4 heads of
  attention, and a row-shard of the output projection (Wo rows for its
  heads) producing a partial [2048,2048] output. Host sums the 4 partials
  per batch.

Key structure:
  - bf16 operands everywhere on the matmul path (same PE rate, half DMA).
  - all large DMAs are split into <=128KB pieces so no single queue
    serializes behind a megabyte transfer (cos/sin were 1MB -> 45us!).
  - scores stay [k, q] (stationary kf k-tile, moving qf), causally
    trimmed on diagonal tiles.
  - exp'd scores (et, bf16) are the STATIONARY operand of the ctx matmul
    [k=128, q=128]; moving operand is V||ones [128, 129] so the softmax
    denominator accumulates as PSUM column 128 -- no ones-matmul, no
    broadcast matmul; normalization is a per-partition DVE tensor_scalar.
  - [q,hd] -> [hd,q] transposes (ctx and V) run on the DMA engines' XBAR
    (16-bit transpose), not the PE/DVE.
  - attention interleaves ctx groups of head h-1 between score tiles of
    head h so the PE never waits on the Activation engine's exp; the
    previous q-chunk's out-projection fills remaining PE slack.
"""

import sys

if "/opt/trn_rl_repo" not in sys.path:
    sys.path.insert(0, "/opt/trn_rl_repo")

from contextlib import ExitStack

import numpy as np
import ml_dtypes

import concourse.bass as bass
import concourse.tile as tile
from concourse import bacc, mybir
from concourse.bass_utils import run_bass_kernel_spmd
from concourse.masks import make_identity

F32 = mybir.dt.float32
F32R = mybir.dt.float32r
BF16 = mybir.dt.bfloat16
AF = mybir.ActivationFunctionType

B, S, D = 2, 2048, 2048
H, G, HD = 16, 4, 128
HPG = H // G          # heads per group = 4
GD = HPG * HD         # group width = 512
P = 128
NCHUNK = 512          # matmul moving free dim
SC = S // NCHUNK      # 4 s-chunks
DT = D // P           # 16 d-tiles
ST = S // P           # 16 s-tiles
SCALE = 1.0 / float(np.sqrt(HD))

_CACHE = {}


def _build():
    nc = bacc.Bacc("TRN2", target_bir_lowering=False, debug=False, num_devices=8)

    # ---- DRAM I/O (per-core shards) ----
    xT = nc.dram_tensor("xT", [D, S], BF16, kind="ExternalInput").ap()
    wq = nc.dram_tensor("wq", [D, GD], BF16, kind="ExternalInput").ap()
    wk = nc.dram_tensor("wk", [D, HD], BF16, kind="ExternalInput").ap()
    wv = nc.dram_tensor("wv", [D, HD], BF16, kind="ExternalInput").ap()
    wo = nc.dram_tensor("wo", [GD, D], BF16, kind="ExternalInput").ap()
    cosT = nc.dram_tensor("cosT", [HD, S], F32, kind="ExternalInput").ap()
    sinT = nc.dram_tensor("sinT", [HD, S], F32, kind="ExternalInput").ap()
    prot = nc.dram_tensor("prot", [HD, HD], F32R, kind="ExternalInput").ap()
    out = nc.dram_tensor("out", [S, D], F32, kind="ExternalOutput").ap()

    xT_v = xT.rearrange("(t p) s -> p t s", p=P)          # [128, 16, 2048]
    wq_v = wq.rearrange("(t p) o -> p t o", p=P)          # [128, 16, 512]
    wk_v = wk.rearrange("(t p) o -> p t o", p=P)          # [128, 16, 128]
    wv_v = wv.rearrange("(t p) o -> p t o", p=P)
    wo_v = wo.rearrange("(h p) d -> p h d", p=P)          # [128, 4, 2048]
    out_v = out.rearrange("(t p) d -> t p d", p=P)        # [16, 128, 2048]

    with tile.TileContext(nc) as tc:
        with ExitStack() as ctx:
            pers = ctx.enter_context(tc.tile_pool(name="pers", bufs=1))
            xpool = ctx.enter_context(tc.tile_pool(name="xpool", bufs=8))
            spool = ctx.enter_context(tc.tile_pool(name="spool", bufs=4))
            epool = ctx.enter_context(tc.tile_pool(name="epool", bufs=34))
            cpool = ctx.enter_context(tc.tile_pool(name="cpool", bufs=2))
            evpool = ctx.enter_context(tc.tile_pool(name="evpool", bufs=6))
            npool = ctx.enter_context(tc.tile_pool(name="npool", bufs=8))
            opool = ctx.enter_context(tc.tile_pool(name="opool", bufs=3))

            # ---- persistent tiles ----
            wq_t = pers.tile([P, DT, GD], BF16, tag="wq")
            wk_t = pers.tile([P, DT, HD], BF16, tag="wk")
            wv_t = pers.tile([P, DT, HD], BF16, tag="wv")
            cos_t = pers.tile([P, S], F32, tag="cos")
            sin_t = pers.tile([P, S], F32, tag="sin")
            prot_t = pers.tile([P, HD], F32R, tag="prot")
            ident = pers.tile([P, P], BF16, tag="ident")
            qf = pers.tile([P, HPG, S], BF16, tag="qf")       # roped Q^T, 4 heads
            kf = pers.tile([P, S], BF16, tag="kf")            # roped K^T
            vnat = pers.tile([P, ST, HD + 1], BF16, tag="vnat")  # V nat + ones col
            wo_t = pers.tile([P, HPG, D], BF16, tag="wo")     # full Wo shard

            # DMA issue costs ~600ns of sequencer time per dma_start, so
            # x rides in dt-pair tiles and issues alternate between the two
            # HWDGE sequencers (SP and Activation). The first matmul's
            # inputs are split in halves so they arrive in ~3us.
            xt0 = []
            for dp in range(DT // 2):
                dt0, dt1 = 2 * dp, 2 * dp + 1
                if dp == 0:
                    for hh in range(2):
                        sl = slice(hh * GD // 2, (hh + 1) * GD // 2)
                        nc.sync.dma_start(wq_t[:, dt0, sl], wq_v[:, dt0, sl])
                    nc.scalar.dma_start(wq_t[:, dt1, :], wq_v[:, dt1, :])
                else:
                    nc.sync.dma_start(wq_t[:, dt0:dt1 + 1, :],
                                      wq_v[:, dt0:dt1 + 1, :])
                nc.scalar.dma_start(wk_t[:, dt0:dt1 + 1, :],
                                    wk_v[:, dt0:dt1 + 1, :])
                nc.scalar.dma_start(wv_t[:, dt0:dt1 + 1, :],
                                    wv_v[:, dt0:dt1 + 1, :])
                xt = xpool.tile([P, 2, NCHUNK], BF16, tag="xt",
                                name=f"xt0_{dp}")
                if dp == 0:
                    for sub in (0, 1):
                        nc.sync.dma_start(xt[:, sub, :],
                                          xT_v[:, 2 * dp + sub, 0:NCHUNK])
                else:
                    eng = nc.sync if dp % 2 == 0 else nc.scalar
                    eng.dma_start(xt[:], xT_v[:, dt0:dt1 + 1, 0:NCHUNK])
                xt0.append(xt)
            for c in range(8):
                sl = slice(c * S // 8, (c + 1) * S // 8)
                nc.scalar.dma_start(cos_t[:, sl], cosT[:, sl])
                nc.scalar.dma_start(sin_t[:, sl], sinT[:, sl])
            nc.sync.dma_start(prot_t[:], prot[:])
            for h in range(HPG):
                nc.scalar.dma_start(wo_t[:, h, :], wo_v[:, h, :])
            make_identity(nc, ident[:])
            nc.gpsimd.memset(vnat[:, :, HD:HD + 1], 1.0)

            # ================= Phase A: QKV projection + RoPE ==============
            with tc.tile_pool(name="psA", bufs=6, space="PSUM") as psA, \
                 tc.tile_pool(name="psAm", bufs=2, space="PSUM") as psAm:

                _m = [0]

                def mbank():
                    _m[0] += 1
                    return psAm.tile([P, NCHUNK], F32, tag="mb",
                                     name=f"mb{_m[0]}")

                def rope(dst, src_sb, sc, mk):
                    """dst[128,512] (bf16) = rope(src_sb [128,512] f32r)."""
                    cs = cos_t[:, sc * NCHUNK:(sc + 1) * NCHUNK]
                    sn = sin_t[:, sc * NCHUNK:(sc + 1) * NCHUNK]
                    rotps = mk()
                    nc.tensor.matmul(rotps[:, :NCHUNK], prot_t[:], src_sb,
                                     start=True, stop=True)
                    t1 = spool.tile([P, NCHUNK], F32, tag="t1")
                    t2 = spool.tile([P, NCHUNK], F32, tag="t2")
                    nc.vector.tensor_mul(t1[:], rotps[:, :NCHUNK], sn)
                    nc.vector.tensor_mul(t2[:], src_sb.bitcast(F32), cs)
                    nc.vector.tensor_add(dst, t2[:], t1[:])

                def phase_a_proj(sc, dps):
                    s0 = sc * NCHUNK
                    for dp in dps:
                        if sc == 0:
                            xtt = xt0[dp]
                        else:
                            xtt = xpool.tile([P, 2, NCHUNK], BF16, tag="xt")
                            eng = nc.sync if dp % 2 == 0 else nc.scalar
                            eng.dma_start(
                                xtt[:],
                                xT_v[:, 2 * dp:2 * dp + 2, s0:s0 + NCHUNK])
                        qps, kps, vps = _acc[sc]
                        for sub in (0, 1):
                            dt = 2 * dp + sub
                            xt = xtt[:, sub, :]
                            st_flag = dt == 0
                            sp_flag = dt == DT - 1
                            for h in range(HPG):
                                nc.tensor.matmul(
                                    qps[h][:],
                                    wq_t[:, dt, h * HD:(h + 1) * HD], xt,
                                    start=st_flag, stop=sp_flag)
                            nc.tensor.matmul(kps[:], wk_t[:, dt, :], xt,
                                             start=st_flag, stop=sp_flag)
                            nc.tensor.matmul(vps[:], wv_t[:, dt, :], xt,
                                             start=st_flag, stop=sp_flag)

                def phase_a_evict(sc):
                    """PSUM -> SBUF evictions, q0 first (next chunk's dt0
                    reads its bank first), alternating ACT/DVE."""
                    qps, kps, vps = _acc[sc]
                    sbs = []
                    for h in range(HPG):
                        qsb = evpool.tile([P, NCHUNK], F32R, tag="ev",
                                          name=f"qsb{sc}_{h}")
                        if h % 2 == 0:
                            nc.scalar.copy(qsb[:], qps[h][:])
                        else:
                            nc.vector.tensor_copy(qsb[:], qps[h][:])
                        sbs.append(qsb)
                    ksb = evpool.tile([P, NCHUNK], F32R, tag="ev",
                                      name=f"ksb{sc}")
                    nc.vector.tensor_copy(ksb[:], kps[:])
                    vsb = evpool.tile([P, NCHUNK], BF16, tag="evb",
                                      name=f"vsb{sc}")
                    nc.scalar.copy(vsb[:], vps[:])
                    return sbs, ksb, vsb

                def phase_a_tail_steps(sc, sbs, ksb, vsb, mk):
                    """RoPE + V^T transpose (PE + DVE, consumes evictions)."""
                    s0 = sc * NCHUNK
                    for h in range(HPG):
                        rope(qf[:, h, s0:s0 + NCHUNK], sbs[h][:], sc, mk)
                        yield
                    rope(kf[:, s0:s0 + NCHUNK], ksb[:], sc, mk)
                    yield
                    for j in range(4):
                        tps = mk().bitcast(BF16)
                        nc.tensor.transpose(
                            tps[:, :P], vsb[:, j * P:(j + 1) * P], ident[:])
                        nc.vector.tensor_copy(
                            vnat[:, sc * 4 + j, 0:HD], tps[:, :P])
                        yield

                def phase_a_tail(sc, sbs, ksb, vsb):
                    for _ in phase_a_tail_steps(sc, sbs, ksb, vsb, mbank):
                        pass

                # The tail (rope rot matmuls) is emitted straight after the
                # evictions: each rot unblocks as its eviction lands, so the
                # PE crosses the chunk boundary without draining.
                _acc = {}
                tail_args = None
                for sc in range(SC):
                    _acc[sc] = ([psA.tile([P, NCHUNK], F32, tag="acc",
                                          name=f"acc{sc}_{i}")
                                 for i in range(HPG)],
                                psA.tile([P, NCHUNK], F32, tag="acc",
                                         name=f"acck{sc}"),
                                psA.tile([P, NCHUNK], F32, tag="acc",
                                         name=f"accv{sc}"))
                    phase_a_proj(sc, range(DT // 2))
                    tail_args = (sc,) + tuple(phase_a_evict(sc))
                    if sc < SC - 1:
                        phase_a_tail(*tail_args)
                # the last chunk's tail is emitted inside the attention
                # scope (it only gates attention q-chunk 3).

            # ======= Phase B/C: attention + out-projection, pipelined ======
            with tc.tile_pool(name="psS", bufs=2, space="PSUM") as psS, \
                 tc.tile_pool(name="psC", bufs=2, space="PSUM") as psC, \
                 tc.tile_pool(name="psT", bufs=2, space="PSUM") as psT, \
                 tc.tile_pool(name="psO", bufs=2, space="PSUM") as psO:

                _s = [0]

                def sbank():
                    _s[0] += 1
                    return psS.tile([P, NCHUNK], F32, tag="sps",
                                    name=f"sps{_s[0]}")

                def attention_steps(qc, ctxq):
                    """Attention for q-chunk qc: per head h, score tiles
                    S(h) stream with ctx groups C(h-1) spread between them;
                    normalized ctx goes to ctxq via DMA XBAR transpose."""
                    q0 = qc * NCHUNK
                    nki = 4 * qc + 4
                    ets = {}
                    pendingT = []
                    _t = [0]

                    def flushT(nleft):
                        while len(pendingT) > nleft:
                            h, j, ctxn = pendingT.pop(0)
                            _t[0] += 1
                            tps = psT.tile([P, NCHUNK], F32, tag="tps",
                                           name=f"tps{qc}_{_t[0]}").bitcast(BF16)
                            nc.tensor.transpose(tps[:, :P], ctxn[:], ident[:])
                            nc.vector.tensor_copy(
                                ctxq[:, h, j * P:(j + 1) * P], tps[:, :P])

                    def S_tile(h, ki):
                        off = max(0, ki - 4 * qc) * P
                        w = NCHUNK - off
                        sps = sbank()
                        nc.tensor.matmul(
                            sps[:, :w], kf[:, ki * P:(ki + 1) * P],
                            qf[:, h, q0 + off:q0 + NCHUNK],
                            start=True, stop=True)
                        et = epool.tile([P, NCHUNK], BF16, tag="et",
                                        name=f"et{qc}_{h}_{ki}")
                        nc.scalar.activation(et[:, :w], sps[:, :w], AF.Exp,
                                             scale=SCALE)
                        if ki >= 4 * qc:
                            # causal mask on the diagonal 128x128 sub-block:
                            # keep where local q (col) >= local k (row)
                            nc.gpsimd.affine_select(
                                out=et[:, 0:P], in_=et[:, 0:P],
                                compare_op=mybir.AluOpType.is_ge,
                                fill=0.0,
                                base=0,
                                channel_multiplier=-1,
                                pattern=[[1, P]],
                            )
                        ets[(h, ki)] = (et, off)

                    def C_group(h, j):
                        qt = 4 * qc + j
                        acc = psC.tile([P, NCHUNK], F32, tag="cacc",
                                       name=f"cacc{qc}_{h}_{j}")
                        for ki in range(qt + 1):
                            et, off = ets[(h, ki)]
                            c0 = j * P - off
                            nc.tensor.matmul(
                                acc[:, :HD + 1], et[:, c0:c0 + P],
                                vnat[:, ki, :],
                                start=(ki == 0), stop=(ki == qt))
                        rec = npool.tile([P, 1], F32, tag="rec")
                        nc.vector.reciprocal_approx_fast(
                            rec[:], acc[:, HD:HD + 1])
                        ctxn = npool.tile([P, P], BF16, tag="ctxn")
                        nc.vector.tensor_scalar_mul(
                            ctxn[:], acc[:, 0:HD], rec[:])
                        pendingT.append((h, j, ctxn))

                    last = qc == SC - 1
                    for h in range(HPG + 1):
                        s_tiles = list(range(nki)) if h < HPG else []
                        c_groups = list(range(4)) if h > 0 else []
                        nc_g = len(c_groups)
                        for i, ki in enumerate(s_tiles):
                            S_tile(h, ki)
                            flushT(2)
                            yield
                            while c_groups and (
                                    len(c_groups) * len(s_tiles) >
                                    nc_g * (len(s_tiles) - 1 - i)):
                                C_group(h - 1, c_groups.pop(0))
                                flushT(2)
                                yield
                        for j in c_groups:
                            C_group(h - 1, j)
                            if h == HPG and last:
                                # final q-chunk: its out-projection rides
                                # right behind each finished q-subtile
                                # instead of draining serially at the end
                                flushT(0)
                                for dc in range(SC):
                                    op_group(qc, ctxq, dc, j)
                                    yield
                            else:
                                flushT(2)
                            yield
                    flushT(0)

                def op_group(qc, ctxq, dc, st):
                    stq = qc * 4 + st
                    ops = psO.tile([P, NCHUNK], F32, tag="ops",
                                   name=f"ops{qc}_{dc}_{st}")
                    for h in range(HPG):
                        nc.tensor.matmul(
                            ops[:], ctxq[:, h, st * P:(st + 1) * P],
                            wo_t[:, h, dc * NCHUNK:(dc + 1) * NCHUNK],
                            start=(h == 0), stop=(h == HPG - 1))
                    osb = opool.tile([P, NCHUNK], F32, tag="osb")
                    if (dc + st) % 2 == 0:
                        nc.vector.tensor_copy(osb[:], ops[:])
                    else:
                        nc.scalar.copy(osb[:], ops[:])
                    nc.sync.dma_start(
                        out_v[stq, :, dc * NCHUNK:(dc + 1) * NCHUNK],
                        osb[:])

                def outproj_steps(qc, ctxq):
                    """Out-projection for q-chunk qc, one (dc, st) group at
                    a time; interleaved into the next chunk's attention."""
                    for dc in range(SC):
                        for st in range(4):
                            op_group(qc, ctxq, dc, st)
                            yield

                # the final s-chunk's rope tail seeds the interleave stream
                # for attention q-chunk 0 (it only gates q-chunk 3).
                out_gen = phase_a_tail_steps(*tail_args, sbank)
                for qc in range(SC):
                    ctxq = cpool.tile([P, HPG, NCHUNK], BF16, tag="ctxq",
                                      name=f"ctxq{qc}")
                    n_steps = 4 * (4 * qc + 4) + 16
                    ratio = max(2, n_steps // 17)
                    k = 0
                    for _ in attention_steps(qc, ctxq):
                        k += 1
                        if out_gen is not None and k % ratio == 0:
                            next(out_gen, None)
                    if out_gen is not None:
                        for _ in out_gen:
                            pass
                    out_gen = (outproj_steps(qc, ctxq)
                               if qc < SC - 1 else None)

    nc.compile()
    return nc


def _host_consts():
    i = np.arange(0, HD, 2, dtype=np.float32)
    inv = (1.0 / (10000.0 ** (i / HD))).astype(np.float32)      # [64]
    t = np.arange(S, dtype=np.float32)
    freqs = t[:, None] * inv[None, :]                           # [S, 64] f32
    emb = np.concatenate([freqs, freqs], axis=1)                # [S, 128]
    cosT = np.cos(emb).T.astype(np.float32).copy()              # [128, S]
    sinT = np.sin(emb).T.astype(np.float32).copy()
    prot = np.zeros((HD, HD), dtype=np.float32)
    half = HD // 2
    for ii in range(half):
        prot[ii + half, ii] = -1.0     # rot[i] = -x[i+64], i < 64
    for ii in range(half, HD):
        prot[ii - half, ii] = 1.0      # rot[i] =  x[i-64], i >= 64
    return cosT, sinT, prot


def _in_maps(x, Wq, Wk, Wv, Wo):
    cosT, sinT, prot = _host_consts()
    bf = ml_dtypes.bfloat16
    # shared per-batch / per-group shards (read-only, safe to alias
    # across the in_maps of the 4 cores that use them)
    xTs = [np.ascontiguousarray(x[b].T.astype(bf)) for b in range(B)]
    wqs = [np.ascontiguousarray(Wq[:, g * GD:(g + 1) * GD].astype(bf))
           for g in range(G)]
    wks = [np.ascontiguousarray(Wk[:, g * HD:(g + 1) * HD].astype(bf))
           for g in range(G)]
    wvs = [np.ascontiguousarray(Wv[:, g * HD:(g + 1) * HD].astype(bf))
           for g in range(G)]
    wos = [np.ascontiguousarray(Wo[g * GD:(g + 1) * GD, :].astype(bf))
           for g in range(G)]
    maps = []
    for c in range(8):
        b, g = c // 4, c % 4
        maps.append({
            "xT": xTs[b], "wq": wqs[g], "wk": wks[g], "wv": wvs[g],
            "wo": wos[g], "cosT": cosT, "sinT": sinT, "prot": prot,
        })
    return maps


def run(x, Wq, Wk, Wv, Wo, trace=False, **trace_kw):
    if "nc" not in _CACHE:
        _CACHE["nc"] = _build()
    nc = _CACHE["nc"]
    maps = _in_maps(
        np.asarray(x, dtype=np.float32), np.asarray(Wq, dtype=np.float32),
        np.asarray(Wk, dtype=np.float32), np.asarray(Wv, dtype=np.float32),
        np.asarray(Wo, dtype=np.float32))
    res = run_bass_kernel_spmd(
        nc, maps, core_ids=list(range(8)), trace=trace, **trace_kw)
    parts = [res.results[c]["out"] for c in range(8)]
    full = np.stack([
        parts[0] + parts[1] + parts[2] + parts[3],
        parts[4] + parts[5] + parts[6] + parts[7],
    ]).astype(np.float32)
    return full, res


def kernel(x, Wq, Wk, Wv, Wo, mask=None):
    full, _ = run(x, Wq, Wk, Wv, Wo, trace=False)
    return full
